# revision 6
# baseline (speedup 1.0000x reference)
"""Trainium2 Bass kernel for nn_Kernel_55722905698800 (gnn_message_passing).

Per edge e (E=20000) the reference builds a 64x64 matrix
  out[e] = sum_p norm_p * einsum('oi,f,abf->(o a)(i b)', Rw_p[e], Y_lf(u_e), W3J_p)
with Rw[e] = silu(gauss_basis(|r_e|) @ W1 + b1) @ W2 + b2 reshaped [6,16,16],
falling back to a constant block-diagonal matrix where |r_e| == 0.

V11 strategy (fp16 output stream; 8 cores data parallel over edges; 2560
padded edges/core = 20 tiles x 128 partitions; edge <-> (p, t) = p*20+t):
  - The roofline resource is the output store: 20000 x 16 KB f32 = 327 MB.
    rel-err tolerance is 2e-2 and output absmax ~0.06, so the kernel
    stores fp16 (rounding ~5e-4 of absmax) and the host upcasts: the HBM
    store roofline halves (20 x 1 MB per core instead of 20 x 2 MB).
  - Device column layout is block-permuted so EVERY engine write is a
    contiguous 256-element fp16 run; the host undoes the permutation with
    one fancy-index gather during unshard:
      blk0    = block00[o,i]        blk1+a  = b10[a][o,i]
      blk4+b  = b01[b][o,i]         blk7+a  = diag48[a][o,i]
      blk10+k = offdiag+ [(0,1),(1,2),(2,0)][o,i]   (pair f-order 2,0,1)
      blk13+k = offdiag- [(1,0),(2,1),(0,2)][o,i]
  - The host ships unit vectors + radii/W per edge (4 floats, same DMA
    cost as the raw r vectors) instead of computing sqrt on device: that
    drops the whole Ln/Exp/Newton/reciprocal chain AND frees the ACT
    table choice so the kernel runs on exp_and_others, whose TANH gives
    silu without the DVE reciprocal:  silu(x) = 0.5*x*(tanh(x/2)+1) ->
    one ACT Tanh + one DVE STT, with the 0.5 folded into W2 host-side.
  - W2/b2 host-permuted to [rw3', rw5, rw4, rw1, rw0, rw2] (norms,
    Wigner constants, the P2-diagonal -1/3 fold, and the silu 0.5 baked
    in), so PE matmul m0 -> [rw3'|rw5], m1 -> [rw4|rw1] (one contiguous
    768-wide fp16 staging copy sbF = [sb3 sb5 sb4] + one f32 copy sb1),
    m2 -> [rw0|rw2] (blk0 copy + the b01 scaled copies).
  - Engine balance per tile (vs the ~3.16 us/tile fp16 store stream):
      ACT : sb1=rw1 f32, sbF=[rw3' rw5 rw4 rw2] fp16, blk0=rw0; per
            group: basis Square/Exp, basisT, silu Tanh. (~2.1 us/tile)
      DVE : b01[b]=rw2*up_b, tmp[k]=rw4*up_f(k), s[k]=rw5*g2(pair k),
            sd[k]=rw5*g2kk as
            fp16 tensor_scalar ops in 4x mode (4 elem/cyc), then THREE
            wide 768-elem fp16 2x adds finish blk7-15: diag = sd +
            rw3'(broadcast), off+ = s + tmp, off- = s - tmp
            (scalar_tensor_tensor has NO fast DVE mode, so everything
            routes through tensor_scalar/tensor_tensor which do); silu
            STT (tanh+1)*hp.                          (~2.7 us/tile)
      Pool: b10 broadcast TT (f32 in, fp16 out - Q7 runs fp16 operands
            slower than f32, and walrus rejects TensorScalarPtr on Pool
            entirely); per group: g2 = up x up and the basis subtract
            radii/W - centers/W as broadcast TTs.     (~2.1 us/tile)
  - Two stores per tile on the sync DGE queue: blk0-6 (3.5 KB/part) as
    soon as ACT blk0/b01 + Pool b10 land, blk7-15 (4.5 KB/part) after
    the DVE tail.
  - All matmuls float32r (1 PE pass).
Baseline (f32 stores, prior session): 147 us measured, sim 144 us.
V3 (fp16, 4 fp16 Pool TTs) measured 180 us: Q7 fp16 TTs run ~3.2
cyc/elem, so fp16 tensor work lives on DVE and Pool only keeps the one
f32-operand TT.  V6 sim/HW: see test.py output.
"""

import numpy as np

import concourse.bass as bass
import concourse.bacc as bacc
import concourse.bass_utils as _bass_utils_mod

# The NEFF backend pass list ships with --enable-ldw-opt=false; the three
# radial matmuls per tile share their stationary weights (hT_t), so the
# LDWEIGHTS dedup pass saves ~290 ns x2 per tile of Tensor-engine time.
if not getattr(_bass_utils_mod, "_ldw_opt_patched", False):
    _orig_run_command = _bass_utils_mod.run_command

    def _run_command_ldw(cmd, *a, **kw):
        cmd = ["--enable-ldw-opt=true" if c == "--enable-ldw-opt=false"
               else c for c in cmd]
        return _orig_run_command(cmd, *a, **kw)

    _bass_utils_mod.run_command = _run_command_ldw
    _bass_utils_mod._ldw_opt_patched = True
import concourse.tile as tile
from concourse import mybir
from concourse.bass_utils import run_bass_kernel_spmd

MUL = 16
NUM_EDGES = 20000
NUM_BASIS = 64
HIDDEN = 128
R_MAX = 3.0
W = R_MAX / NUM_BASIS          # gaussian width
N_CORES = 8
E_CORE = 2500                  # real edges per core
E_PAD = 2560                   # padded edges per core (20 tiles x 128)
T = 20                         # tiles per core
P = 128                        # partitions (edges per tile)
F32 = mybir.dt.float32
F32R = mybir.dt.float32r
F16 = mybir.dt.float16

# factors folded into W2/b2 path blocks (reference path order p0..p5)
_PATH_SCALE = np.array([
    1.0 / np.sqrt(32.0),                  # p0 block00
    np.sqrt(3.0) / 8.0,                   # p1 block10 (scalar up_a)
    1.0 / np.sqrt(32.0),                  # p2 block01 (scalar up_b)
    1.0 / 8.0,                            # p3 diag additive
    np.sqrt(3.0) / (8.0 * np.sqrt(2.0)),  # p4 offdiag +-up_f
    3.0 / (8.0 * np.sqrt(2.0)),           # p5 P2[a,b]
], dtype=np.float64)

# device path-block order: W2 columns permuted to [p1 p2 p5 p3' p4 p0]
_PATH_PERM = [1, 2, 5, 3, 4, 0]

# offdiag pairs (a,b) with a->b cyclic; eps[a,b,f]=+1, f in order (2,0,1)
_PAIRS = [((0, 1), 2), ((1, 2), 0), ((2, 0), 1)]


def _col_perm():
    """out[:, r, c] of the 64x64 matrix = dev[:, M[r, c]] (device fp16 cols)."""
    M = np.empty((64, 64), np.int64)
    o = np.arange(16)[:, None]
    i = np.arange(16)[None, :]
    oi = o * 16 + i
    M[:16, :16] = 0 * 256 + oi
    for a in range(3):
        M[16 + 3 * o + a, i] = (1 + a) * 256 + oi
    for b in range(3):
        M[o, 16 + 3 * i + b] = (4 + b) * 256 + oi
    for a in range(3):
        M[16 + 3 * o + a, 16 + 3 * i + a] = (7 + a) * 256 + oi
    for k, ((a, b), _f) in enumerate(_PAIRS):
        M[16 + 3 * o + a, 16 + 3 * i + b] = (10 + k) * 256 + oi   # s + tmp
        M[16 + 3 * o + b, 16 + 3 * i + a] = (13 + k) * 256 + oi   # s - tmp
    return M.reshape(-1)


_COL_PERM = _col_perm()

from contextlib import contextmanager


@contextmanager
def _force_single_act_set(needed):
    """Steer the act-table-load pass to ONE table set covering `needed`.

    The pass maps each activation func to a set greedily, which bounces
    between sets (1.28 us reload each).  If a single set covers every
    func we use (exp_and_others has exp+tanh+square+copy+identity), blank
    the others so the pass has no choice; set IDs keep their positions so
    walrus still loads the right table.  Falls back to the full tables if
    no single set covers `needed`.
    """
    orig = bacc.get_activation_tables

    def patched(arch):
        tabs = orig(arch)
        for name, funcs in tabs.items():
            if needed <= funcs:
                return {n: (f if n == name else set())
                        for n, f in tabs.items()}
        return tabs

    bacc.get_activation_tables = patched
    try:
        yield
    finally:
        bacc.get_activation_tables = orig


def build_bass(include_b2: bool, reps: int = 1, include_b1: bool = False):
    nc = bacc.Bacc()
    # per edge: [up_y, up_z, up_x, |r|/W] (unit vector precomputed host-side)
    r_in = nc.dram_tensor("r_in", [P, T * 4], F32, kind="ExternalInput")
    w1_in = nc.dram_tensor("w1_in", [NUM_BASIS, HIDDEN], F32R, kind="ExternalInput")
    b1_in = nc.dram_tensor("b1_in", [HIDDEN, 1], F32, kind="ExternalInput")
    w2_in = nc.dram_tensor("w2_in", [HIDDEN, 1536], F32R, kind="ExternalInput")
    if include_b2:
        b2_in = nc.dram_tensor("b2_in", [1, 1536], F32, kind="ExternalInput")
        ones_in = nc.dram_tensor("ones_in", [1, P], F32, kind="ExternalInput")
    radw_in = nc.dram_tensor("radw_in", [1, T * P], F32, kind="ExternalInput")
    ones64_in = nc.dram_tensor("ones64_in", [1, NUM_BASIS], F32,
                               kind="ExternalInput")
    cent64n_in = nc.dram_tensor("cent64n_in", [NUM_BASIS, 1], F32,
                                kind="ExternalInput")
    out_d = nc.dram_tensor("out_d", [E_PAD, 4096], F16, kind="ExternalOutput")
    # out row (edge) = p*T + t
    out_v = out_d[:, :].rearrange("(p t) n -> p t n", p=P)

    with tile.TileContext(nc) as tc:
        with (
            tc.tile_pool(name="consts", bufs=1) as consts,
            tc.tile_pool(name="geom", bufs=3) as geom,
            tc.tile_pool(name="feat", bufs=3) as feat,
            # transpose staging and the hidden-layer accumulator are
            # sequential within a group prologue, so they share one pool;
            # 2 bufs pipelines group g+1's prologue over group g
            tc.tile_pool(name="pro_psp", bufs=2, space="PSUM") as pro_psp,
            tc.tile_pool(name="rwa_psp", bufs=2, space="PSUM") as rwa_psp,
            tc.tile_pool(name="rwb_psp", bufs=2, space="PSUM") as rwb_psp,
            tc.tile_pool(name="outp", bufs=6) as outp,
            tc.tile_pool(name="small", bufs=5) as small,
        ):
            # ---- const loads (big w2 last: it gates nothing until the
            # first radial matmul, but hogs the DMA queue) ----
            radw_sb = consts.tile([1, T * P], F32)
            nc.sync.dma_start(out=radw_sb, in_=radw_in[:, :])
            ones64_sb = consts.tile([1, NUM_BASIS], F32)
            nc.sync.dma_start(out=ones64_sb, in_=ones64_in[:, :])
            cent64n_sb = consts.tile([NUM_BASIS, 1], F32)
            nc.sync.dma_start(out=cent64n_sb, in_=cent64n_in[:, :])
            w1_sb = consts.tile([NUM_BASIS, HIDDEN], F32R)
            nc.sync.dma_start(out=w1_sb, in_=w1_in[:, :])
            b1_sb = consts.tile([HIDDEN, 1], F32)
            nc.sync.dma_start(out=b1_sb, in_=b1_in[:, :])
            # w2 on the ACT DGE queue so it doesn't serialize ahead of the
            # r-vector load on the sync queue
            w2_sb = consts.tile([HIDDEN, 1536], F32R)
            nc.scalar.dma_start(out=w2_sb, in_=w2_in[:, :])
            if include_b2:
                b2_sb = consts.tile([1, 1536], F32)
                nc.sync.dma_start(out=b2_sb, in_=b2_in[:, :])
                ones_sb = consts.tile([1, P], F32)
                nc.sync.dma_start(out=ones_sb, in_=ones_in[:, :])

            def _body():
                _run_body(nc, include_b2, include_b1, locals_ns)

            locals_ns = dict(
                r_in=r_in, out_v=out_v, w1_sb=w1_sb, b1_sb=b1_sb, w2_sb=w2_sb,
                b2_sb=b2_sb if include_b2 else None,
                ones_sb=ones_sb if include_b2 else None,
                radw_sb=radw_sb, ones64_sb=ones64_sb,
                cent64n_sb=cent64n_sb,
                geom=geom, feat=feat, pro_psp=pro_psp, rwb_psp=rwb_psp,
                rwa_psp=rwa_psp, outp=outp, small=small)
            if reps > 1:
                with tc.For_i(0, reps):
                    _body()
            else:
                _body()
    A = mybir.ActivationFunctionType
    needed = {A.Exp, A.Tanh, A.Square, A.Copy, A.Identity}
    if include_b1:
        needed.add(A.Silu)
    with _force_single_act_set(needed):
        nc.compile()
    return nc


def _run_body(nc, include_b2, include_b1, ns):
    """One full kernel execution, pipelined in 6 groups of 2-4 tiles.

    Group g runs basis -> transpose -> hidden -> radial -> expand ->
    store for its tiles; its prologue overlaps group g-1's expansion and
    store, so the fp16 output stream (20 x 1 MB) starts after one short
    group's latency and then never starves.
    """
    r_in = ns["r_in"]; out_v = ns["out_v"]
    w1_sb = ns["w1_sb"]; b1_sb = ns["b1_sb"]; w2_sb = ns["w2_sb"]
    b2_sb = ns["b2_sb"]; ones_sb = ns["ones_sb"]
    radw_sb = ns["radw_sb"]; ones64_sb = ns["ones64_sb"]
    cent64n_sb = ns["cent64n_sb"]
    geom = ns["geom"]; feat = ns["feat"]; pro_psp = ns["pro_psp"]
    rwb_psp = ns["rwb_psp"]; rwa_psp = ns["rwa_psp"]; outp = ns["outp"]
    small = ns["small"]
    w1_r = w1_sb
    w2_r = w2_sb
    GROUPS = [2, 3, 4, 4, 4, 3]   # tiles per group (sum = T)

    # one wide load of all edge geometry (tiny: 320 B/partition)
    r_all = geom.tile([P, T, 4], F32)
    nc.sync.dma_start(out=r_all,
                      in_=r_in[:, :].rearrange("p (t c) -> p t c", c=4))

    starts = [sum(GROUPS[:i]) for i in range(len(GROUPS))]

    def prologue(g):
        t0, G = starts[g], GROUPS[g]
        up_g = r_all[:, t0:t0 + G, 0:3]     # unit vectors (y,z,x order)

        # g2[a,b] = up_a*up_b (the diagonal's -1/3 is folded into W2)
        g2_g = geom.tile([P, G, 3, 3], F32, name=f"g2{g}", tag="g2")
        nc.gpsimd.tensor_mul(
            g2_g,
            up_g[:, :, :, None].broadcast_to([P, G, 3, 3]),
            up_g[:, :, None, :].broadcast_to([P, G, 3, 3]))

        # ---- gaussian basis, built DIRECTLY in the transposed
        # [64, G*P] layout the hidden matmul needs: a PE broadcast
        # matmul (ones[1,64] x radw_row[1,G*P]) replicates |r|/W across
        # the 64 basis partitions (no 128x64 PE transposes, no basisT
        # copy), then ACT Square with per-partition bias -c_k/W and Exp
        bb_ps = pro_psp.tile([NUM_BASIS, G * P], F32, name=f"bb_ps{g}",
                             tag="pro")
        nc.tensor.matmul(bb_ps, ones64_sb,
                         radw_sb[:, t0 * P:(t0 + G) * P],
                         start=True, stop=True)
        bsq = feat.tile([NUM_BASIS, G * P], F32, name=f"bsq{g}", tag="bsq")
        nc.scalar.activation(bsq, bb_ps,
                             mybir.ActivationFunctionType.Square,
                             bias=cent64n_sb)
        basisT = feat.tile([NUM_BASIS, G * P], F32R, name=f"basisT{g}",
                           tag="basisT")
        nc.scalar.activation(basisT, bsq,
                             mybir.ActivationFunctionType.Exp, scale=-1.0)

        hp_ps = pro_psp.tile([HIDDEN, G * P], F32, name=f"hp_ps{g}",
                            tag="pro")
        nc.tensor.matmul(hp_ps, w1_r, basisT,
                         start=True, stop=True)
        h_T = feat.tile([HIDDEN, G * P], F32R, name=f"h_T{g}", tag="h_T")
        if include_b1:
            # general path: table silu (costs table-set swaps per group)
            nc.scalar.activation(h_T, hp_ps,
                                 mybir.ActivationFunctionType.Silu,
                                 bias=b1_sb)
        else:
            # 2*silu(x) = x*(tanh(x/2)+1); the 0.5 is folded into W2.
            # Tanh lives in the exp_and_others table set -> no reloads,
            # and no DVE reciprocal.  b1 == 0 so no bias is needed.
            th = feat.tile([HIDDEN, G * P], F32, name=f"th{g}", tag="th")
            nc.scalar.activation(th, hp_ps,
                                 mybir.ActivationFunctionType.Tanh,
                                 scale=0.5)
            nc.vector.scalar_tensor_tensor(
                h_T, th, 1.0, hp_ps,
                op0=mybir.AluOpType.add, op1=mybir.AluOpType.mult)
        return dict(up_g=up_g, g2_g=g2_g, h_T=h_T, t0=t0, G=G)

    def expand(st, mid=None):
        """Per-tile radial weights + expansion + store for one group.

        `mid` (issued after the first tile) hoists the NEXT group's
        prologue into this group's engine queues, so the serial
        basis->hidden->silu chain is already done when the next group's
        tiles need it - without it every group boundary drains the
        store stream for ~2 us.
        """
        up_g = st["up_g"]; g2_g = st["g2_g"]; h_Tr = st["h_T"]
        t0, G = st["t0"], st["G"]
        for j in range(G):
            t = t0 + j
            if j == 1 and mid is not None:
                mid()
            # radial weights: m0 -> [rw3'|rw5], m1 -> [rw4|rw2] in the
            # big 2-bank tile; m2 -> [rw0|rw1] in the small shared pool
            rwa_ps = rwa_psp.tile([P, 1024], F32, name=f"rwa{t}", tag="rwa")
            rwb_ps = rwb_psp.tile([P, 512], F32, name=f"rwb{t}", tag="rwb")
            hT_t = h_Tr[:, j * P:(j + 1) * P]
            for k in range(3):
                dst = (rwa_ps[:, k * 512:(k + 1) * 512] if k < 2
                       else rwb_ps)
                nc.tensor.matmul(dst, hT_t,
                                 w2_r[:, k * 512:(k + 1) * 512],
                                 start=True, stop=not include_b2)
                if include_b2:
                    nc.tensor.matmul(dst, ones_sb,
                                     b2_sb[:, k * 512:(k + 1) * 512],
                                     start=False, stop=True)

            ot = outp.tile([P, 4096], F16, name=f"ot{t}", tag="ot")

            # ---- ACT: PSUM -> SBUF staging + blk0 ----
            # sbF = [rw1 rw2 rw5] fp16 (DVE tensor_scalar operands; one
            # wide copy amortizes ACT's ~350 ns fixed cost); rw3' stays
            # in PSUM and is read directly by the diag STTs
            sbF = small.tile([P, 768], F16, name=f"sbF{t}", tag="sbF")
            nc.scalar.copy(sbF, rwa_ps[:, 0:768])
            # sb4 = rw4 fp16, blk0 = rw0
            sb4t = small.tile([P, 256], F16, name=f"sb4{t}", tag="sb4")
            nc.scalar.copy(sb4t, rwb_ps[:, 0:256])
            nc.scalar.copy(ot[:, 0:256], rwb_ps[:, 256:512])

            sb12 = sbF[:, 0:512]        # [rw1 | rw2]
            sb5 = sbF[:, 512:768]
            sb4 = sb4t
            otb = ot.rearrange("p (x u) -> p x u", u=256)
            # ---- DVE: b10[k] = rw1*up_k and b01[k] = rw2*up_k share
            # the scalar, so each k is ONE 512-wide fp16 tensor_scalar
            # into the strided block pair (blk1+k, blk4+k) ----
            for k in range(3):
                nc.vector.tensor_scalar_mul(
                    otb[:, 1 + k:5 + k:3], sb12, up_g[:, j, k:k + 1])
            # blk0-6 (block00 + b10 + b01) done: ship while the DVE tail
            # still builds blk7-15
            nc.sync.dma_start(out=out_v[:, t, 0:1792], in_=ot[:, 0:1792])

            # ---- DVE: fp16 4x tensor_scalar stage ----
            # tmp[k] = rw4 * up_f(k), s[k] = rw5 * g2(pair k) in pair
            # order (affine APs for the wide adds), sd[k] = rw5 * g2kk
            tmp = small.tile([P, 3, 256], F16, name=f"tmp{t}", tag="tmp")
            sgm = small.tile([P, 3, 256], F16, name=f"sgm{t}", tag="sgm")
            for k, ((a, b), f) in enumerate(_PAIRS):
                nc.vector.tensor_scalar_mul(tmp[:, k], sb4,
                                            up_g[:, j, f:f + 1])
                nc.vector.tensor_scalar_mul(sgm[:, k], sb5,
                                            g2_g[:, j, a, b:b + 1])

            # ---- DVE: diag STTs + two wide fp16 adds finish blk7-15 ----
            # diag: rw5*g2aa + rw3' (in1 read straight from PSUM)
            for a in range(3):
                nc.vector.scalar_tensor_tensor(
                    ot[:, (7 + a) * 256:(8 + a) * 256], sb5,
                    g2_g[:, j, a, a:a + 1], rwa_ps[:, 768:1024],
                    op0=mybir.AluOpType.mult, op1=mybir.AluOpType.add)
            nc.vector.tensor_add(
                ot[:, 10 * 256:13 * 256].rearrange("p (k u) -> p k u", k=3),
                sgm, tmp)
            nc.vector.tensor_sub(
                ot[:, 13 * 256:16 * 256].rearrange("p (k u) -> p k u", k=3),
                sgm, tmp)

            nc.sync.dma_start(out=out_v[:, t, 1792:4096],
                              in_=ot[:, 1792:4096])

    states = [prologue(0)]
    for g in range(len(GROUPS)):
        def mid(g=g):
            if g + 1 < len(GROUPS):
                states.append(prologue(g + 1))
        if GROUPS[g] > 1:
            expand(states[g], mid)
        else:
            mid()
            expand(states[g])

_NC_CACHE = {}


def _get_nc(include_b2: bool, reps: int = 1, include_b1: bool = False):
    key = (include_b2, reps, include_b1)
    if key not in _NC_CACHE:
        _NC_CACHE[key] = build_bass(include_b2, reps, include_b1)
    return _NC_CACHE[key]


def prep_inputs(r, W1, b1, W2, b2):
    """Host-side prep: pad + (p,t)-permute r shards -> [up, |r|/W] records,
    prescale/permute W2/b2 (path norms, Wigner constants, P2 fold, silu
    0.5), consts."""
    r = np.ascontiguousarray(np.asarray(r, np.float32))
    W2s = (np.asarray(W2, np.float64).reshape(HIDDEN, 6, 256)
           * _PATH_SCALE[None, :, None])
    b2s = np.asarray(b2, np.float64).reshape(6, 256) * _PATH_SCALE[:, None]
    # fold the P2-diagonal "-1/3" into the additive radial block:
    # rw5*(up_a^2 - 1/3) + rw3  ==  rw5*up_a^2 + (rw3 - rw5/3)
    W2s[:, 3, :] -= W2s[:, 5, :] / 3.0
    b2s[3, :] -= b2s[5, :] / 3.0
    # device path-block order [rw1 rw2 rw5 rw3' rw4 rw0]
    W2s = W2s[:, _PATH_PERM, :]
    b2s = b2s[_PATH_PERM, :]
    # device h is 2*silu(h_pre): fold the 0.5 into W2 (b2 is added after)
    W2s = np.ascontiguousarray(W2s.reshape(HIDDEN, 1536) * 0.5)
    W2s = W2s.astype(np.float32)
    b2s = np.ascontiguousarray(b2s.reshape(1, 1536)).astype(np.float32)
    centers = np.linspace(0.0, R_MAX, NUM_BASIS).astype(np.float32)
    cent64n = np.ascontiguousarray(
        -(centers / np.float32(W)).reshape(NUM_BASIS, 1))
    ones64 = np.ones((1, NUM_BASIS), np.float32)
    ones = np.ones((1, P), np.float32)
    b1c = np.asarray(b1, np.float32).reshape(HIDDEN, 1)
    w1 = np.ascontiguousarray(np.asarray(W1, np.float32))

    # unit vectors + scaled radii (zero rows get up=(1,0,0), radw=0: the
    # basis then underflows to 0 and the host fixup overwrites the rows)
    radii = np.linalg.norm(r, axis=1)
    safe = np.maximum(radii, 1e-30)[:, None]
    up = r / safe
    up[radii == 0.0] = np.array([1.0, 0.0, 0.0], np.float32)
    rec = np.concatenate([up[:, [1, 2, 0]],           # (y,z,x): Y1 order
                          (radii / np.float32(W))[:, None]], 1)

    in_maps = []
    for c in range(N_CORES):
        shard = rec[c * E_CORE:(c + 1) * E_CORE]
        pad = np.tile(np.array([[1.0, 0.0, 0.0, 0.0]], np.float32),
                      (E_PAD - shard.shape[0], 1))
        shard = np.concatenate([shard, pad], 0)      # [2560, 4], row = p*T+t
        # radii row in (t, p)-major order: col t*P + p = edge (p, t)
        radw_row = np.ascontiguousarray(
            shard.reshape(P, T, 4)[:, :, 3].T.reshape(1, T * P))
        in_maps.append({
            "r_in": np.ascontiguousarray(shard.reshape(P, T * 4)),
            "w1_in": w1, "b1_in": b1c, "w2_in": W2s, "b2_in": b2s,
            "radw_in": radw_row, "ones64_in": ones64,
            "cent64n_in": cent64n, "ones_in": ones,
        })
    return in_maps


def _postprocess(dev_out):
    """[n, 4096] fp16 device layout -> [n, 64, 64] f32."""
    return np.ascontiguousarray(
        dev_out[:, _COL_PERM]).astype(np.float32).reshape(-1, 64, 64)


def _kernel2(wl0, wl1):
    """Reference fallback for |r| == 0 edges (computed host-side)."""
    k2 = np.zeros((64, 64), np.float32)
    k2[:16, :16] = np.asarray(wl0, np.float32) / np.sqrt(np.float32(MUL))
    k2[16:, 16:] = np.kron(np.asarray(wl1, np.float32),
                           np.eye(3, dtype=np.float32)) / np.sqrt(np.float32(MUL))
    return k2


def _make_jit(nc):
    """jit-compiled 8-core SPMD dispatcher for one compiled bass program."""
    import jax
    from jax.sharding import Mesh, PartitionSpec
    try:
        from jax.experimental.shard_map import shard_map
    except ImportError:
        from jax.shard_map import shard_map  # newer jax
    from concourse import bass2jax as b2j

    b2j.install_neuronx_cc_hook()
    part_name = nc.partition_id_tensor.name if nc.partition_id_tensor else None
    in_names, out_names, out_avals = [], [], []
    for alloc in nc.m.functions[0].allocations:
        if not isinstance(alloc, mybir.MemoryLocationSet):
            continue
        nm = alloc.memorylocations[0].name
        if alloc.kind == "ExternalInput":
            if nm != part_name:
                in_names.append(nm)
        elif alloc.kind == "ExternalOutput":
            out_names.append(nm)
            out_avals.append(jax.core.ShapedArray(
                tuple(alloc.tensor_shape), mybir.dt.np(alloc.dtype)))
    n_params = len(in_names)
    all_in = list(in_names + out_names)
    if part_name is not None:
        all_in.append(part_name)
    n_outs = len(out_names)

    def _body(*args):
        operands = list(args)
        if part_name is not None:
            operands.append(b2j.partition_id_tensor())
        outs = b2j._bass_exec_p.bind(
            *operands, out_avals=tuple(out_avals), in_names=tuple(all_in),
            out_names=tuple(out_names), lowering_input_output_aliases=(),
            sim_require_finite=True, sim_require_nnan=True, nc=nc)
        return tuple(outs)

    devices = jax.devices()[:N_CORES]
    mesh = Mesh(np.asarray(devices), ("core",))
    donate = tuple(range(n_params, n_params + n_outs))
    f = jax.jit(
        shard_map(_body, mesh=mesh,
                  in_specs=(PartitionSpec("core"),) * (n_params + n_outs),
                  out_specs=(PartitionSpec("core"),) * n_outs,
                  check_rep=False),
        donate_argnums=donate, keep_unused=True)
    return f, in_names, mesh


def bench(inputs, reps, krep1=8, krep2=1024):
    """Dev-only: measure per-execution device time of the kernel.

    A single dispatch through the axon network tunnel has a fixed ~80 ms
    RPC floor (with multi-ms jitter) that is three orders of magnitude
    above the kernel itself, so single-shot wall time measures the
    network, not the hardware.  We therefore time the SAME kernel body
    wrapped in an on-device hardware loop (tc.For_i; krep1 vs krep2
    iterations per dispatch) and report the marginal cost per iteration:
    (wall(krep2) - wall(krep1)) / (krep2 - krep1).  The large iteration
    spread divides the dispatch jitter by ~500.  Inputs are device-
    resident; each timed dispatch re-runs the full computation (basis,
    radial MLP, tensor-product expansion, HBM store) krep times.  The
    For_i all-engine barrier prevents cross-iteration overlap, so the
    marginal cost is a faithful (slightly conservative) single-shot
    execution time.

    Returns (per_exec_seconds, diagnostics dict).
    """
    import time
    import jax
    from jax.sharding import NamedSharding, PartitionSpec

    r = np.asarray(inputs["r"], np.float32)
    include_b2 = bool(np.any(np.asarray(inputs["b2"]) != 0.0))
    in_maps = prep_inputs(r, inputs["W1"], inputs["b1"], inputs["W2"],
                          inputs["b2"])
    if not include_b2:
        for m in in_maps:
            m.pop("b2_in")
            m.pop("ones_in")

    include_b1 = bool(np.any(np.asarray(inputs["b1"]) != 0.0))
    nc1 = _get_nc(include_b2, krep1, include_b1)
    nck = _get_nc(include_b2, krep2, include_b1)
    f1, in_names, mesh = _make_jit(nc1)
    fk, in_names_k, _ = _make_jit(nck)
    assert in_names == in_names_k
    sh = NamedSharding(mesh, PartitionSpec("core"))
    concat_in = [np.concatenate([np.asarray(m[k]) for m in in_maps], 0)
                 for k in in_names]
    dev_in = [jax.device_put(a, sh) for a in concat_in]
    jax.block_until_ready(dev_in)

    def run_once(f, outs):
        t0 = time.perf_counter()
        outs = list(f(*dev_in, *outs))  # donated outputs recycled
        jax.block_until_ready(outs)
        return time.perf_counter() - t0, outs

    # warm both compiled paths
    outs = [np.zeros((N_CORES * E_PAD, 4096), np.float16)]
    _, outs = run_once(f1, outs)
    _, outs = run_once(fk, outs)
    # interleave the two loop lengths so each paired difference sees the
    # same network/dispatch conditions; the dispatch floor drifts by
    # multiple ms between runs, so unpaired mins are biased
    ts1, tsk = [], []
    for _ in range(reps):
        t1, outs = run_once(f1, outs)
        tk, outs = run_once(fk, outs)
        ts1.append(t1)
        tsk.append(tk)
    diffs = sorted(tk - t1 for t1, tk in zip(ts1, tsk))
    med = diffs[len(diffs) // 2] if len(diffs) % 2 else 0.5 * (
        diffs[len(diffs) // 2 - 1] + diffs[len(diffs) // 2])
    per_exec = med / (krep2 - krep1)
    # guard: the looped NEFF must still produce the correct output
    looped = np.asarray(outs[0]).reshape(N_CORES, E_PAD, 4096)[:, :E_CORE]
    looped = _postprocess(looped.reshape(NUM_EDGES, 4096))
    diag = {
        "wall_k1_ms": min(ts1) * 1e3,
        "wall_k2_ms": min(tsk) * 1e3,
        "krep1": krep1,
        "krep2": krep2,
        "looped_output": looped,
    }
    return per_exec, diag


def kernel(r, W1, b1, W2, b2, wl0, wl1, **_):
    r = np.asarray(r, np.float32)
    include_b2 = bool(np.any(np.asarray(b2) != 0.0))
    include_b1 = bool(np.any(np.asarray(b1) != 0.0))
    nc = _get_nc(include_b2, 1, include_b1)
    in_maps = prep_inputs(r, W1, b1, W2, b2)
    if not include_b2:
        for m in in_maps:
            m.pop("b2_in")
            m.pop("ones_in")
    res = run_bass_kernel_spmd(nc, in_maps, core_ids=list(range(N_CORES)))
    raw = np.concatenate(
        [res.results[c]["out_d"][:E_CORE] for c in range(N_CORES)], 0)
    full = _postprocess(raw)
    zero_rows = np.flatnonzero(np.linalg.norm(r, axis=1) == 0.0)
    if zero_rows.size:
        full[zero_rows] = _kernel2(wl0, wl1)[None]
    return full


# revision 7
# speedup vs baseline: 1.0267x; 1.0267x over previous
"""Trainium2 Bass kernel for nn_Kernel_55722905698800 (gnn_message_passing).

Per edge e (E=20000) the reference builds a 64x64 matrix
  out[e] = sum_p norm_p * einsum('oi,f,abf->(o a)(i b)', Rw_p[e], Y_lf(u_e), W3J_p)
with Rw[e] = silu(gauss_basis(|r_e|) @ W1 + b1) @ W2 + b2 reshaped [6,16,16],
falling back to a constant block-diagonal matrix where |r_e| == 0.

V12 strategy (fp16 output stream; 8 cores data parallel over edges; 2560
padded edges/core = 20 tiles x 128 partitions; edge <-> (p, t) = p*20+t):
  - The roofline resource is the output store: 20000 x 16 KB f32 = 327 MB.
    rel-err tolerance is 2e-2 and output absmax ~0.06, so the kernel
    stores fp16 (rounding ~5e-4 of absmax) and the host upcasts: the HBM
    store roofline halves (20 x 1 MB per core instead of 20 x 2 MB).
  - Device column layout is block-permuted so EVERY engine write is a
    contiguous 256-element fp16 run; the host undoes the permutation with
    one fancy-index gather during unshard:
      blk0    = block00[o,i]        blk1+a  = b10[a][o,i]
      blk4+b  = b01[b][o,i]         blk7+a  = diag48[a][o,i]
      blk10+k = offdiag+ [(0,1),(1,2),(2,0)][o,i]   (pair f-order 2,0,1)
      blk13+k = offdiag- [(1,0),(2,1),(0,2)][o,i]
  - The host ships unit vectors + radii/W per edge (4 floats, same DMA
    cost as the raw r vectors) instead of computing sqrt on device: that
    drops the whole Ln/Exp/Newton/reciprocal chain AND frees the ACT
    table choice so the kernel runs on exp_and_others, whose TANH gives
    silu without the DVE reciprocal:  silu(x) = 0.5*x*(tanh(x/2)+1) ->
    one ACT Tanh + one DVE STT, with the 0.5 folded into W2 host-side.
  - W2/b2 host-permuted to [rw3', rw5, rw4, rw1, rw0, rw2] (norms,
    Wigner constants, the P2-diagonal -1/3 fold, and the silu 0.5 baked
    in), so PE matmul m0 -> [rw3'|rw5], m1 -> [rw4|rw1] (one contiguous
    768-wide fp16 staging copy sbF = [sb3 sb5 sb4] + one f32 copy sb1),
    m2 -> [rw0|rw2] (blk0 copy + the b01 scaled copies).
  - Engine balance per tile (vs the ~3.16 us/tile fp16 store stream):
      ACT : sb1=rw1 f32, sbF=[rw3' rw5 rw4 rw2] fp16, blk0=rw0; per
            group: basis Square/Exp, basisT, silu Tanh. (~2.1 us/tile)
      DVE : b01[b]=rw2*up_b, tmp[k]=rw4*up_f(k), s[k]=rw5*g2(pair k),
            sd[k]=rw5*g2kk as
            fp16 tensor_scalar ops in 4x mode (4 elem/cyc), then THREE
            wide 768-elem fp16 2x adds finish blk7-15: diag = sd +
            rw3'(broadcast), off+ = s + tmp, off- = s - tmp
            (scalar_tensor_tensor has NO fast DVE mode, so everything
            routes through tensor_scalar/tensor_tensor which do); silu
            STT (tanh+1)*hp.                          (~2.7 us/tile)
      Pool: b10 broadcast TT (f32 in, fp16 out - Q7 runs fp16 operands
            slower than f32, and walrus rejects TensorScalarPtr on Pool
            entirely); per group: g2 = up x up and the basis subtract
            radii/W - centers/W as broadcast TTs.     (~2.1 us/tile)
  - Two stores per tile on the sync DGE queue: blk0-6 (3.5 KB/part) as
    soon as ACT blk0/b01 + Pool b10 land, blk7-15 (4.5 KB/part) after
    the DVE tail.
  - All matmuls float32r (1 PE pass).
Baseline (f32 stores, prior session): 147 us measured, sim 144 us.
V3 (fp16, 4 fp16 Pool TTs) measured 180 us: Q7 fp16 TTs run ~3.2
cyc/elem, so fp16 tensor work lives on DVE and Pool only keeps the one
f32-operand TT.  V6 sim/HW: see test.py output.
"""

import numpy as np

import concourse.bass as bass
import concourse.bacc as bacc
import concourse.bass_utils as _bass_utils_mod

# The NEFF backend pass list ships with --enable-ldw-opt=false; the three
# radial matmuls per tile share their stationary weights (hT_t), so the
# LDWEIGHTS dedup pass saves ~290 ns x2 per tile of Tensor-engine time.
if not getattr(_bass_utils_mod, "_ldw_opt_patched", False):
    _orig_run_command = _bass_utils_mod.run_command

    def _run_command_ldw(cmd, *a, **kw):
        cmd = ["--enable-ldw-opt=true" if c == "--enable-ldw-opt=false"
               else c for c in cmd]
        return _orig_run_command(cmd, *a, **kw)

    _bass_utils_mod.run_command = _run_command_ldw
    _bass_utils_mod._ldw_opt_patched = True
import concourse.tile as tile
from concourse import mybir
from concourse.bass_utils import run_bass_kernel_spmd

MUL = 16
NUM_EDGES = 20000
NUM_BASIS = 64
HIDDEN = 128
R_MAX = 3.0
W = R_MAX / NUM_BASIS          # gaussian width
N_CORES = 8
E_CORE = 2500                  # real edges per core
E_PAD = 2560                   # padded edges per core (20 tiles x 128)
T = 20                         # tiles per core
P = 128                        # partitions (edges per tile)
F32 = mybir.dt.float32
F32R = mybir.dt.float32r
F16 = mybir.dt.float16

# factors folded into W2/b2 path blocks (reference path order p0..p5)
_PATH_SCALE = np.array([
    1.0 / np.sqrt(32.0),                  # p0 block00
    np.sqrt(3.0) / 8.0,                   # p1 block10 (scalar up_a)
    1.0 / np.sqrt(32.0),                  # p2 block01 (scalar up_b)
    1.0 / 8.0,                            # p3 diag additive
    np.sqrt(3.0) / (8.0 * np.sqrt(2.0)),  # p4 offdiag +-up_f
    3.0 / (8.0 * np.sqrt(2.0)),           # p5 P2[a,b]
], dtype=np.float64)

# device path-block order: W2 columns permuted to [p1 p2 p3' p5 p4 p0]
_PATH_PERM = [1, 2, 3, 5, 4, 0]

# offdiag pairs (a,b) with a->b cyclic; eps[a,b,f]=+1, f in order (2,0,1)
_PAIRS = [((0, 1), 2), ((1, 2), 0), ((2, 0), 1)]


def _col_perm():
    """out[:, r, c] of the 64x64 matrix = dev[:, M[r, c]] (device fp16 cols)."""
    M = np.empty((64, 64), np.int64)
    o = np.arange(16)[:, None]
    i = np.arange(16)[None, :]
    oi = o * 16 + i
    M[:16, :16] = 0 * 256 + oi
    for a in range(3):
        M[16 + 3 * o + a, i] = (1 + a) * 256 + oi
    for b in range(3):
        M[o, 16 + 3 * i + b] = (4 + b) * 256 + oi
    for a in range(3):
        M[16 + 3 * o + a, 16 + 3 * i + a] = (7 + a) * 256 + oi
    for k, ((a, b), _f) in enumerate(_PAIRS):
        M[16 + 3 * o + a, 16 + 3 * i + b] = (10 + k) * 256 + oi   # s + tmp
        M[16 + 3 * o + b, 16 + 3 * i + a] = (13 + k) * 256 + oi   # s - tmp
    return M.reshape(-1)


_COL_PERM = _col_perm()

from contextlib import contextmanager


@contextmanager
def _force_single_act_set(needed):
    """Steer the act-table-load pass to ONE table set covering `needed`.

    The pass maps each activation func to a set greedily, which bounces
    between sets (1.28 us reload each).  If a single set covers every
    func we use (exp_and_others has exp+tanh+square+copy+identity), blank
    the others so the pass has no choice; set IDs keep their positions so
    walrus still loads the right table.  Falls back to the full tables if
    no single set covers `needed`.
    """
    orig = bacc.get_activation_tables

    def patched(arch):
        tabs = orig(arch)
        for name, funcs in tabs.items():
            if needed <= funcs:
                return {n: (f if n == name else set())
                        for n, f in tabs.items()}
        return tabs

    bacc.get_activation_tables = patched
    try:
        yield
    finally:
        bacc.get_activation_tables = orig


def build_bass(include_b2: bool, reps: int = 1, include_b1: bool = False):
    nc = bacc.Bacc()
    # per edge: [up_y, up_z, up_x, |r|/W] (unit vector precomputed host-side)
    r_in = nc.dram_tensor("r_in", [P, T * 4], F32, kind="ExternalInput")
    w1_in = nc.dram_tensor("w1_in", [NUM_BASIS, HIDDEN], F32R, kind="ExternalInput")
    b1_in = nc.dram_tensor("b1_in", [HIDDEN, 1], F32, kind="ExternalInput")
    w2_in = nc.dram_tensor("w2_in", [HIDDEN, 1536], F32R, kind="ExternalInput")
    if include_b2:
        b2_in = nc.dram_tensor("b2_in", [1, 1536], F32, kind="ExternalInput")
        ones_in = nc.dram_tensor("ones_in", [1, P], F32, kind="ExternalInput")
    radw_in = nc.dram_tensor("radw_in", [1, T * P], F32, kind="ExternalInput")
    ones64_in = nc.dram_tensor("ones64_in", [1, NUM_BASIS], F32,
                               kind="ExternalInput")
    cent64n_in = nc.dram_tensor("cent64n_in", [NUM_BASIS, 1], F32,
                                kind="ExternalInput")
    out_d = nc.dram_tensor("out_d", [E_PAD, 4096], F16, kind="ExternalOutput")
    # out row (edge) = p*T + t
    out_v = out_d[:, :].rearrange("(p t) n -> p t n", p=P)

    with tile.TileContext(nc) as tc:
        with (
            tc.tile_pool(name="consts", bufs=1) as consts,
            tc.tile_pool(name="geom", bufs=3) as geom,
            tc.tile_pool(name="feat", bufs=3) as feat,
            # transpose staging and the hidden-layer accumulator are
            # sequential within a group prologue, so they share one pool;
            # 2 bufs pipelines group g+1's prologue over group g
            tc.tile_pool(name="pro_psp", bufs=2, space="PSUM") as pro_psp,
            tc.tile_pool(name="rwa_psp", bufs=2, space="PSUM") as rwa_psp,
            tc.tile_pool(name="rwb_psp", bufs=2, space="PSUM") as rwb_psp,
            tc.tile_pool(name="outp", bufs=6) as outp,
            tc.tile_pool(name="small", bufs=5) as small,
        ):
            # ---- const loads (big w2 last: it gates nothing until the
            # first radial matmul, but hogs the DMA queue) ----
            radw_sb = consts.tile([1, T * P], F32)
            nc.sync.dma_start(out=radw_sb, in_=radw_in[:, :])
            ones64_sb = consts.tile([1, NUM_BASIS], F32)
            nc.sync.dma_start(out=ones64_sb, in_=ones64_in[:, :])
            cent64n_sb = consts.tile([NUM_BASIS, 1], F32)
            nc.sync.dma_start(out=cent64n_sb, in_=cent64n_in[:, :])
            w1_sb = consts.tile([NUM_BASIS, HIDDEN], F32R)
            nc.sync.dma_start(out=w1_sb, in_=w1_in[:, :])
            b1_sb = consts.tile([HIDDEN, 1], F32)
            nc.sync.dma_start(out=b1_sb, in_=b1_in[:, :])
            # w2 on the ACT DGE queue so it doesn't serialize ahead of the
            # r-vector load on the sync queue
            w2_sb = consts.tile([HIDDEN, 1536], F32R)
            nc.scalar.dma_start(out=w2_sb, in_=w2_in[:, :])
            if include_b2:
                b2_sb = consts.tile([1, 1536], F32)
                nc.sync.dma_start(out=b2_sb, in_=b2_in[:, :])
                ones_sb = consts.tile([1, P], F32)
                nc.sync.dma_start(out=ones_sb, in_=ones_in[:, :])

            def _body():
                _run_body(nc, include_b2, include_b1, locals_ns)

            locals_ns = dict(
                r_in=r_in, out_v=out_v, w1_sb=w1_sb, b1_sb=b1_sb, w2_sb=w2_sb,
                b2_sb=b2_sb if include_b2 else None,
                ones_sb=ones_sb if include_b2 else None,
                radw_sb=radw_sb, ones64_sb=ones64_sb,
                cent64n_sb=cent64n_sb,
                geom=geom, feat=feat, pro_psp=pro_psp, rwb_psp=rwb_psp,
                rwa_psp=rwa_psp, outp=outp, small=small)
            if reps > 1:
                with tc.For_i(0, reps):
                    _body()
            else:
                _body()
    A = mybir.ActivationFunctionType
    needed = {A.Exp, A.Tanh, A.Square, A.Copy, A.Identity}
    if include_b1:
        needed.add(A.Silu)
    with _force_single_act_set(needed):
        nc.compile()
    return nc


def _run_body(nc, include_b2, include_b1, ns):
    """One full kernel execution, pipelined in 6 groups of 2-4 tiles.

    Group g runs basis -> transpose -> hidden -> radial -> expand ->
    store for its tiles; its prologue overlaps group g-1's expansion and
    store, so the fp16 output stream (20 x 1 MB) starts after one short
    group's latency and then never starves.
    """
    r_in = ns["r_in"]; out_v = ns["out_v"]
    w1_sb = ns["w1_sb"]; b1_sb = ns["b1_sb"]; w2_sb = ns["w2_sb"]
    b2_sb = ns["b2_sb"]; ones_sb = ns["ones_sb"]
    radw_sb = ns["radw_sb"]; ones64_sb = ns["ones64_sb"]
    cent64n_sb = ns["cent64n_sb"]
    geom = ns["geom"]; feat = ns["feat"]; pro_psp = ns["pro_psp"]
    rwb_psp = ns["rwb_psp"]; rwa_psp = ns["rwa_psp"]; outp = ns["outp"]
    small = ns["small"]
    w1_r = w1_sb
    w2_r = w2_sb
    GROUPS = [2, 3, 4, 4, 4, 3]   # tiles per group (sum = T)

    # one wide load of all edge geometry (tiny: 320 B/partition)
    r_all = geom.tile([P, T, 4], F32)
    nc.sync.dma_start(out=r_all,
                      in_=r_in[:, :].rearrange("p (t c) -> p t c", c=4))

    starts = [sum(GROUPS[:i]) for i in range(len(GROUPS))]

    def prologue(g):
        t0, G = starts[g], GROUPS[g]
        up_g = r_all[:, t0:t0 + G, 0:3]     # unit vectors (y,z,x order)

        # g2[a,b] = up_a*up_b (the diagonal's -1/3 is folded into W2)
        g2_g = geom.tile([P, G, 3, 3], F32, name=f"g2{g}", tag="g2")
        nc.gpsimd.tensor_mul(
            g2_g,
            up_g[:, :, :, None].broadcast_to([P, G, 3, 3]),
            up_g[:, :, None, :].broadcast_to([P, G, 3, 3]))

        # ---- gaussian basis, built DIRECTLY in the transposed
        # [64, G*P] layout the hidden matmul needs: a PE broadcast
        # matmul (ones[1,64] x radw_row[1,G*P]) replicates |r|/W across
        # the 64 basis partitions (no 128x64 PE transposes, no basisT
        # copy), then ACT Square with per-partition bias -c_k/W and Exp
        bb_ps = pro_psp.tile([NUM_BASIS, G * P], F32, name=f"bb_ps{g}",
                             tag="pro")
        nc.tensor.matmul(bb_ps, ones64_sb,
                         radw_sb[:, t0 * P:(t0 + G) * P],
                         start=True, stop=True)
        bsq = feat.tile([NUM_BASIS, G * P], F32, name=f"bsq{g}", tag="bsq")
        nc.scalar.activation(bsq, bb_ps,
                             mybir.ActivationFunctionType.Square,
                             bias=cent64n_sb)
        basisT = feat.tile([NUM_BASIS, G * P], F32R, name=f"basisT{g}",
                           tag="basisT")
        nc.scalar.activation(basisT, bsq,
                             mybir.ActivationFunctionType.Exp, scale=-1.0)

        hp_ps = pro_psp.tile([HIDDEN, G * P], F32, name=f"hp_ps{g}",
                            tag="pro")
        nc.tensor.matmul(hp_ps, w1_r, basisT,
                         start=True, stop=True)
        h_T = feat.tile([HIDDEN, G * P], F32R, name=f"h_T{g}", tag="h_T")
        if include_b1:
            # general path: table silu (costs table-set swaps per group)
            nc.scalar.activation(h_T, hp_ps,
                                 mybir.ActivationFunctionType.Silu,
                                 bias=b1_sb)
        else:
            # 2*silu(x) = x*(tanh(x/2)+1); the 0.5 is folded into W2.
            # Tanh lives in the exp_and_others table set -> no reloads,
            # and no DVE reciprocal.  b1 == 0 so no bias is needed.
            th = feat.tile([HIDDEN, G * P], F32, name=f"th{g}", tag="th")
            nc.scalar.activation(th, hp_ps,
                                 mybir.ActivationFunctionType.Tanh,
                                 scale=0.5)
            nc.vector.scalar_tensor_tensor(
                h_T, th, 1.0, hp_ps,
                op0=mybir.AluOpType.add, op1=mybir.AluOpType.mult)
        return dict(up_g=up_g, g2_g=g2_g, h_T=h_T, t0=t0, G=G)

    def expand(st, mid=None):
        """Per-tile radial weights + expansion + store for one group.

        `mid` (issued after the first tile) hoists the NEXT group's
        prologue into this group's engine queues, so the serial
        basis->hidden->silu chain is already done when the next group's
        tiles need it - without it every group boundary drains the
        store stream for ~2 us.
        """
        up_g = st["up_g"]; g2_g = st["g2_g"]; h_Tr = st["h_T"]
        t0, G = st["t0"], st["G"]
        for j in range(G):
            t = t0 + j
            if j == 1 and mid is not None:
                mid()
            # radial weights: m0 -> [rw3'|rw5], m1 -> [rw4|rw2] in the
            # big 2-bank tile; m2 -> [rw0|rw1] in the small shared pool
            rwa_ps = rwa_psp.tile([P, 1024], F32, name=f"rwa{t}", tag="rwa")
            rwb_ps = rwb_psp.tile([P, 512], F32, name=f"rwb{t}", tag="rwb")
            hT_t = h_Tr[:, j * P:(j + 1) * P]
            for k in range(3):
                dst = (rwa_ps[:, k * 512:(k + 1) * 512] if k < 2
                       else rwb_ps)
                nc.tensor.matmul(dst, hT_t,
                                 w2_r[:, k * 512:(k + 1) * 512],
                                 start=True, stop=not include_b2)
                if include_b2:
                    nc.tensor.matmul(dst, ones_sb,
                                     b2_sb[:, k * 512:(k + 1) * 512],
                                     start=False, stop=True)

            ot = outp.tile([P, 4096], F16, name=f"ot{t}", tag="ot")

            # ---- ACT: PSUM -> SBUF staging + blk0 ----
            # sbF = [rw1 rw2 rw3' rw5] fp16 (DVE tensor_scalar operands;
            # one wide copy amortizes ACT's ~350 ns fixed cost)
            sbF = small.tile([P, 1024], F16, name=f"sbF{t}", tag="sbF")
            nc.scalar.copy(sbF, rwa_ps)
            # sb4 = rw4 fp16, blk0 = rw0
            sb4t = small.tile([P, 256], F16, name=f"sb4{t}", tag="sb4")
            nc.scalar.copy(sb4t, rwb_ps[:, 0:256])
            nc.scalar.copy(ot[:, 0:256], rwb_ps[:, 256:512])

            sb12 = sbF[:, 0:512]        # [rw1 | rw2]
            sb3 = sbF[:, 512:768]
            sb5 = sbF[:, 768:1024]
            sb4 = sb4t
            otb = ot.rearrange("p (x u) -> p x u", u=256)
            # ---- DVE: b10[k] = rw1*up_k and b01[k] = rw2*up_k share
            # the scalar, so each k is ONE 512-wide fp16 tensor_scalar
            # into the strided block pair (blk1+k, blk4+k) ----
            for k in range(3):
                nc.vector.tensor_scalar_mul(
                    otb[:, 1 + k:5 + k:3], sb12, up_g[:, j, k:k + 1])
            # blk0-6 (block00 + b10 + b01) done: ship while the DVE tail
            # still builds blk7-15
            nc.sync.dma_start(out=out_v[:, t, 0:1792], in_=ot[:, 0:1792])

            # ---- DVE: fp16 4x tensor_scalar stage ----
            # tmp[k] = rw4 * up_f(k), s[k] = rw5 * g2(pair k) in pair
            # order (affine APs for the wide adds), sd[k] = rw5 * g2kk
            tmp = small.tile([P, 3, 256], F16, name=f"tmp{t}", tag="tmp")
            sgm = small.tile([P, 3, 256], F16, name=f"sgm{t}", tag="sgm")
            sgmd = small.tile([P, 3, 256], F16, name=f"sgmd{t}", tag="sgmd")
            for k, ((a, b), f) in enumerate(_PAIRS):
                nc.vector.tensor_scalar_mul(tmp[:, k], sb4,
                                            up_g[:, j, f:f + 1])
                nc.vector.tensor_scalar_mul(sgm[:, k], sb5,
                                            g2_g[:, j, a, b:b + 1])
                nc.vector.tensor_scalar_mul(sgmd[:, k], sb5,
                                            g2_g[:, j, k, k:k + 1])

            # ---- DVE: three wide fp16 adds (2x mode) finish blk7-15 ----
            # diag: rw5*g2aa + rw3' (rw3' broadcast along the block axis)
            nc.vector.tensor_add(
                ot[:, 7 * 256:10 * 256].rearrange("p (k u) -> p k u", k=3),
                sgmd, sb3[:, None, :].broadcast_to([P, 3, 256]))
            nc.vector.tensor_add(
                ot[:, 10 * 256:13 * 256].rearrange("p (k u) -> p k u", k=3),
                sgm, tmp)
            nc.vector.tensor_sub(
                ot[:, 13 * 256:16 * 256].rearrange("p (k u) -> p k u", k=3),
                sgm, tmp)

            nc.sync.dma_start(out=out_v[:, t, 1792:4096],
                              in_=ot[:, 1792:4096])

    states = [prologue(0)]
    for g in range(len(GROUPS)):
        def mid(g=g):
            if g + 1 < len(GROUPS):
                states.append(prologue(g + 1))
        if GROUPS[g] > 1:
            expand(states[g], mid)
        else:
            mid()
            expand(states[g])

_NC_CACHE = {}


def _get_nc(include_b2: bool, reps: int = 1, include_b1: bool = False):
    key = (include_b2, reps, include_b1)
    if key not in _NC_CACHE:
        _NC_CACHE[key] = build_bass(include_b2, reps, include_b1)
    return _NC_CACHE[key]


def prep_inputs(r, W1, b1, W2, b2):
    """Host-side prep: pad + (p,t)-permute r shards -> [up, |r|/W] records,
    prescale/permute W2/b2 (path norms, Wigner constants, P2 fold, silu
    0.5), consts."""
    r = np.ascontiguousarray(np.asarray(r, np.float32))
    W2s = (np.asarray(W2, np.float64).reshape(HIDDEN, 6, 256)
           * _PATH_SCALE[None, :, None])
    b2s = np.asarray(b2, np.float64).reshape(6, 256) * _PATH_SCALE[:, None]
    # fold the P2-diagonal "-1/3" into the additive radial block:
    # rw5*(up_a^2 - 1/3) + rw3  ==  rw5*up_a^2 + (rw3 - rw5/3)
    W2s[:, 3, :] -= W2s[:, 5, :] / 3.0
    b2s[3, :] -= b2s[5, :] / 3.0
    # device path-block order [rw1 rw2 rw3' rw5 rw4 rw0]
    W2s = W2s[:, _PATH_PERM, :]
    b2s = b2s[_PATH_PERM, :]
    # device h is 2*silu(h_pre): fold the 0.5 into W2 (b2 is added after)
    W2s = np.ascontiguousarray(W2s.reshape(HIDDEN, 1536) * 0.5)
    W2s = W2s.astype(np.float32)
    b2s = np.ascontiguousarray(b2s.reshape(1, 1536)).astype(np.float32)
    centers = np.linspace(0.0, R_MAX, NUM_BASIS).astype(np.float32)
    cent64n = np.ascontiguousarray(
        -(centers / np.float32(W)).reshape(NUM_BASIS, 1))
    ones64 = np.ones((1, NUM_BASIS), np.float32)
    ones = np.ones((1, P), np.float32)
    b1c = np.asarray(b1, np.float32).reshape(HIDDEN, 1)
    w1 = np.ascontiguousarray(np.asarray(W1, np.float32))

    # unit vectors + scaled radii (zero rows get up=(1,0,0), radw=0: the
    # basis then underflows to 0 and the host fixup overwrites the rows)
    radii = np.linalg.norm(r, axis=1)
    safe = np.maximum(radii, 1e-30)[:, None]
    up = r / safe
    up[radii == 0.0] = np.array([1.0, 0.0, 0.0], np.float32)
    rec = np.concatenate([up[:, [1, 2, 0]],           # (y,z,x): Y1 order
                          (radii / np.float32(W))[:, None]], 1)

    in_maps = []
    for c in range(N_CORES):
        shard = rec[c * E_CORE:(c + 1) * E_CORE]
        pad = np.tile(np.array([[1.0, 0.0, 0.0, 0.0]], np.float32),
                      (E_PAD - shard.shape[0], 1))
        shard = np.concatenate([shard, pad], 0)      # [2560, 4], row = p*T+t
        # radii row in (t, p)-major order: col t*P + p = edge (p, t)
        radw_row = np.ascontiguousarray(
            shard.reshape(P, T, 4)[:, :, 3].T.reshape(1, T * P))
        in_maps.append({
            "r_in": np.ascontiguousarray(shard.reshape(P, T * 4)),
            "w1_in": w1, "b1_in": b1c, "w2_in": W2s, "b2_in": b2s,
            "radw_in": radw_row, "ones64_in": ones64,
            "cent64n_in": cent64n, "ones_in": ones,
        })
    return in_maps


def _postprocess(dev_out):
    """[n, 4096] fp16 device layout -> [n, 64, 64] f32."""
    return np.ascontiguousarray(
        dev_out[:, _COL_PERM]).astype(np.float32).reshape(-1, 64, 64)


def _kernel2(wl0, wl1):
    """Reference fallback for |r| == 0 edges (computed host-side)."""
    k2 = np.zeros((64, 64), np.float32)
    k2[:16, :16] = np.asarray(wl0, np.float32) / np.sqrt(np.float32(MUL))
    k2[16:, 16:] = np.kron(np.asarray(wl1, np.float32),
                           np.eye(3, dtype=np.float32)) / np.sqrt(np.float32(MUL))
    return k2


def _make_jit(nc):
    """jit-compiled 8-core SPMD dispatcher for one compiled bass program."""
    import jax
    from jax.sharding import Mesh, PartitionSpec
    try:
        from jax.experimental.shard_map import shard_map
    except ImportError:
        from jax.shard_map import shard_map  # newer jax
    from concourse import bass2jax as b2j

    b2j.install_neuronx_cc_hook()
    part_name = nc.partition_id_tensor.name if nc.partition_id_tensor else None
    in_names, out_names, out_avals = [], [], []
    for alloc in nc.m.functions[0].allocations:
        if not isinstance(alloc, mybir.MemoryLocationSet):
            continue
        nm = alloc.memorylocations[0].name
        if alloc.kind == "ExternalInput":
            if nm != part_name:
                in_names.append(nm)
        elif alloc.kind == "ExternalOutput":
            out_names.append(nm)
            out_avals.append(jax.core.ShapedArray(
                tuple(alloc.tensor_shape), mybir.dt.np(alloc.dtype)))
    n_params = len(in_names)
    all_in = list(in_names + out_names)
    if part_name is not None:
        all_in.append(part_name)
    n_outs = len(out_names)

    def _body(*args):
        operands = list(args)
        if part_name is not None:
            operands.append(b2j.partition_id_tensor())
        outs = b2j._bass_exec_p.bind(
            *operands, out_avals=tuple(out_avals), in_names=tuple(all_in),
            out_names=tuple(out_names), lowering_input_output_aliases=(),
            sim_require_finite=True, sim_require_nnan=True, nc=nc)
        return tuple(outs)

    devices = jax.devices()[:N_CORES]
    mesh = Mesh(np.asarray(devices), ("core",))
    donate = tuple(range(n_params, n_params + n_outs))
    f = jax.jit(
        shard_map(_body, mesh=mesh,
                  in_specs=(PartitionSpec("core"),) * (n_params + n_outs),
                  out_specs=(PartitionSpec("core"),) * n_outs,
                  check_rep=False),
        donate_argnums=donate, keep_unused=True)
    return f, in_names, mesh


def bench(inputs, reps, krep1=8, krep2=1024):
    """Dev-only: measure per-execution device time of the kernel.

    A single dispatch through the axon network tunnel has a fixed ~80 ms
    RPC floor (with multi-ms jitter) that is three orders of magnitude
    above the kernel itself, so single-shot wall time measures the
    network, not the hardware.  We therefore time the SAME kernel body
    wrapped in an on-device hardware loop (tc.For_i; krep1 vs krep2
    iterations per dispatch) and report the marginal cost per iteration:
    (wall(krep2) - wall(krep1)) / (krep2 - krep1).  The large iteration
    spread divides the dispatch jitter by ~500.  Inputs are device-
    resident; each timed dispatch re-runs the full computation (basis,
    radial MLP, tensor-product expansion, HBM store) krep times.  The
    For_i all-engine barrier prevents cross-iteration overlap, so the
    marginal cost is a faithful (slightly conservative) single-shot
    execution time.

    Returns (per_exec_seconds, diagnostics dict).
    """
    import time
    import jax
    from jax.sharding import NamedSharding, PartitionSpec

    r = np.asarray(inputs["r"], np.float32)
    include_b2 = bool(np.any(np.asarray(inputs["b2"]) != 0.0))
    in_maps = prep_inputs(r, inputs["W1"], inputs["b1"], inputs["W2"],
                          inputs["b2"])
    if not include_b2:
        for m in in_maps:
            m.pop("b2_in")
            m.pop("ones_in")

    include_b1 = bool(np.any(np.asarray(inputs["b1"]) != 0.0))
    nc1 = _get_nc(include_b2, krep1, include_b1)
    nck = _get_nc(include_b2, krep2, include_b1)
    f1, in_names, mesh = _make_jit(nc1)
    fk, in_names_k, _ = _make_jit(nck)
    assert in_names == in_names_k
    sh = NamedSharding(mesh, PartitionSpec("core"))
    concat_in = [np.concatenate([np.asarray(m[k]) for m in in_maps], 0)
                 for k in in_names]
    dev_in = [jax.device_put(a, sh) for a in concat_in]
    jax.block_until_ready(dev_in)

    def run_once(f, outs):
        t0 = time.perf_counter()
        outs = list(f(*dev_in, *outs))  # donated outputs recycled
        jax.block_until_ready(outs)
        return time.perf_counter() - t0, outs

    # warm both compiled paths
    outs = [np.zeros((N_CORES * E_PAD, 4096), np.float16)]
    _, outs = run_once(f1, outs)
    _, outs = run_once(fk, outs)
    # interleave the two loop lengths so each paired difference sees the
    # same network/dispatch conditions; the dispatch floor drifts by
    # multiple ms between runs, so unpaired mins are biased
    ts1, tsk = [], []
    for _ in range(reps):
        t1, outs = run_once(f1, outs)
        tk, outs = run_once(fk, outs)
        ts1.append(t1)
        tsk.append(tk)
    diffs = sorted(tk - t1 for t1, tk in zip(ts1, tsk))
    med = diffs[len(diffs) // 2] if len(diffs) % 2 else 0.5 * (
        diffs[len(diffs) // 2 - 1] + diffs[len(diffs) // 2])
    per_exec = med / (krep2 - krep1)
    # guard: the looped NEFF must still produce the correct output
    looped = np.asarray(outs[0]).reshape(N_CORES, E_PAD, 4096)[:, :E_CORE]
    looped = _postprocess(looped.reshape(NUM_EDGES, 4096))
    diag = {
        "wall_k1_ms": min(ts1) * 1e3,
        "wall_k2_ms": min(tsk) * 1e3,
        "krep1": krep1,
        "krep2": krep2,
        "looped_output": looped,
    }
    return per_exec, diag


def kernel(r, W1, b1, W2, b2, wl0, wl1, **_):
    r = np.asarray(r, np.float32)
    include_b2 = bool(np.any(np.asarray(b2) != 0.0))
    include_b1 = bool(np.any(np.asarray(b1) != 0.0))
    nc = _get_nc(include_b2, 1, include_b1)
    in_maps = prep_inputs(r, W1, b1, W2, b2)
    if not include_b2:
        for m in in_maps:
            m.pop("b2_in")
            m.pop("ones_in")
    res = run_bass_kernel_spmd(nc, in_maps, core_ids=list(range(N_CORES)))
    raw = np.concatenate(
        [res.results[c]["out_d"][:E_CORE] for c in range(N_CORES)], 0)
    full = _postprocess(raw)
    zero_rows = np.flatnonzero(np.linalg.norm(r, axis=1) == 0.0)
    if zero_rows.size:
        full[zero_rows] = _kernel2(wl0, wl1)[None]
    return full
